# revision 20
# baseline (speedup 1.0000x reference)
import sys

sys.path.insert(0, "/opt/trn_rl_repo")
import numpy as np

B, S, D, H, R = 2, 2048, 768, 12, 16
LORA_SCALE = 1.0 / R
W = D // H  # 64
HPC = 3  # heads per core
WPC = HPC * W  # 192 output dims per core
NCORES = 8
SB = 512  # s-block for projections
NT = S // 128  # 16 t-chunks

_cache = {}


def _build():
    import concourse.bacc as bacc
    import concourse.mybir as mybir
    import concourse.tile as tile

    f32 = mybir.dt.float32
    bf16 = mybir.dt.bfloat16
    AF = mybir.ActivationFunctionType

    nc = bacc.Bacc("TRN2", target_bir_lowering=False, debug=False)
    xT_d = nc.dram_tensor("xT", [D, S], bf16, kind="ExternalInput")
    WAT_d = nc.dram_tensor("WAT", [D, 432], bf16, kind="ExternalInput")
    WvT_d = nc.dram_tensor("WvT", [D, WPC], bf16, kind="ExternalInput")
    BqT_d = nc.dram_tensor("BqT", [R, WPC], bf16, kind="ExternalInput")
    BvT_d = nc.dram_tensor("BvT", [R, WPC], bf16, kind="ExternalInput")
    bias_d = nc.dram_tensor("bias_qk", [128, 4], f32, kind="ExternalInput")
    bv_d = nc.dram_tensor("bv_row", [1, WPC], bf16, kind="ExternalInput")
    mb_d = nc.dram_tensor("mb", [128, NT], f32, kind="ExternalInput")
    out_d = nc.dram_tensor("outT", [HPC * 65, S], f32, kind="ExternalOutput")

    with tile.TileContext(nc) as tc:
        with tc.tile_pool(name="cst", bufs=1) as cst:
            xT = cst.tile([128, 6, S], bf16, name="xT")
            WAT = cst.tile([128, 6, 432], bf16, name="WAT")
            WvT = cst.tile([128, 6, WPC], bf16, name="WvT")
            BqT = cst.tile([R, WPC], bf16, name="BqT")
            BvT = cst.tile([49, WPC], bf16, name="BvT")
            bias = cst.tile([128, 4], f32, name="bias")
            mb = cst.tile([128, NT], f32, name="mb")
            QT = cst.tile([128, 2, S], bf16, name="QT")
            KT = cst.tile([128, 2, S], bf16, name="KT")
            u = cst.tile([49, S], bf16, name="u")  # 0:16 uq, 32:48 uv, 48 ones
            V = cst.tile([128, NT, 195], bf16, name="V")
            OT = [cst.tile([65, S], f32, name=f"ot{h}") for h in range(HPC)]

            nc.sync.dma_start(xT[:], xT_d.ap().rearrange("(c p) s -> p c s", p=128))
            nc.sync.dma_start(WAT[:], WAT_d.ap().rearrange("(c p) m -> p c m", p=128))
            nc.sync.dma_start(WvT[:], WvT_d.ap().rearrange("(c p) m -> p c m", p=128))
            nc.gpsimd.dma_start(BqT[:], BqT_d.ap())
            nc.gpsimd.dma_start(BvT[32:48, :], BvT_d.ap())
            nc.gpsimd.dma_start(bias[:], bias_d.ap())
            nc.gpsimd.dma_start(BvT[48:49, :], bv_d.ap())
            nc.vector.memset(u[32:49, :], 1.0)
            nc.gpsimd.dma_start(mb[:], mb_d.ap())
            nc.vector.memset(V[:, :, 64::65], 1.0)
            tc.strict_bb_all_engine_barrier()

            # ---- phase 1: projections ----
            import os
            LVL = int(os.environ.get("P1LVL", "9"))
            # W_A cols: q 0:192 | k 192:384 | Aq 384:400 | pad | Av 416:432
            chunk_cols = [(0, 128), (128, 192), (192, 320), (320, 384)]
            drains = [
                (QT, 0, 128, 0), (QT, 1, 64, 1), (KT, 0, 128, 2), (KT, 1, 64, 3),
            ]
            with (
                tc.tile_pool(name="pu0", bufs=1, space="PSUM") as pu_pool,
                tc.tile_pool(name="pc0", bufs=1, space="PSUM") as pc0,
                tc.tile_pool(name="pc1", bufs=1, space="PSUM") as pc1,
                tc.tile_pool(name="pc2", bufs=1, space="PSUM") as pc2,
                tc.tile_pool(name="pc3", bufs=1, space="PSUM") as pc3,
                tc.tile_pool(name="vpa", bufs=1, space="PSUM") as vpa,
                tc.tile_pool(name="vpb", bufs=1, space="PSUM") as vpb,
            ):
                pc = [pc0, pc1, pc2, pc3]
                for sb in range(S // SB if LVL >= 1 else 0):
                    ssl = slice(sb * SB, (sb + 1) * SB)
                    pu = pu_pool.tile([48, SB], f32, name="pu")
                    for c in range(6):
                        nc.tensor.matmul(
                            pu[:], WAT[:, c, 384:432], xT[:, c, ssl],
                            start=(c == 0), stop=(c == 5),
                        )
                    nc.vector.tensor_copy(u[0:48, ssl], pu[:])
                    for ci in range(4 if LVL >= 2 else 0):
                        c0, c1 = chunk_cols[ci]
                        m = c1 - c0
                        p = pc[ci].tile([128, SB], f32, name=f"pc{ci}t")
                        has_lora = ci < 2
                        if has_lora:
                            nc.tensor.matmul(
                                p[:m], BqT[:, c0:c1], u[0:16, ssl],
                                start=True, stop=False, skip_group_check=True,
                            )
                        for c in range(6):
                            nc.tensor.matmul(
                                p[:m], WAT[:, c, c0:c1], xT[:, c, ssl],
                                start=(c == 0 and not has_lora), stop=(c == 5),
                                skip_group_check=True,
                            )
                        dst, di, dm, bc = drains[ci]
                        nc.vector.tensor_scalar_add(
                            dst[0:dm, di, ssl], p[0:dm], bias[0:dm, bc:bc + 1]
                        )

                # V: normal layout [s, w] per 128-chunk
                for t in range(NT if LVL >= 3 else 0):
                    tsl = slice(t * 128, (t + 1) * 128)
                    p = (vpa if t % 2 == 0 else vpb).tile([128, WPC], f32, name="vpt")
                    nc.tensor.matmul(p[:], u[32:49, tsl], BvT[32:49, :], start=True,
                                     stop=False, skip_group_check=True)
                    for c in range(6):
                        nc.tensor.matmul(
                            p[:], xT[:, c, tsl], WvT[:, c, :],
                            start=False, stop=(c == 5), skip_group_check=True,
                        )
                    for hh in range(HPC):
                        nc.vector.tensor_copy(V[:, t, hh * 65:hh * 65 + 64],
                                              p[:, hh * 64:(hh + 1) * 64])

            # ---- phase 2: attention ----
            import os
            if os.environ.get("PHASE1_ONLY"):
                for h in range(HPC):
                    nc.gpsimd.dma_start(out_d.ap()[h * 65:(h + 1) * 65, :], QT[0:65, 0, :])
                phase2 = False
            else:
                phase2 = True
            qk_src = [(QT, 0, 0), (QT, 0, 64), (QT, 1, 0)]
            with (
                tc.tile_pool(name="sp", bufs=1, space="PSUM") as sp,
                tc.tile_pool(name="op", bufs=1, space="PSUM") as op,
                tc.tile_pool(name="pt", bufs=2) as ptp,
            ):
                for h in range(HPC if phase2 else 0):
                    _, ci, pb = qk_src[h]
                    q_ap = QT[pb:pb + 64, ci, :]
                    outp = op.tile([65, S], f32, name="op")
                    for t in range(NT):
                        spt = sp.tile([128, S], f32, name="sp")
                        for nb in range(S // 512):
                            nsl = slice(nb * 512, (nb + 1) * 512)
                            nc.tensor.matmul(
                                spt[:, nsl], KT[pb:pb + 64, ci, t * 128:(t + 1) * 128],
                                q_ap[:, nsl], start=True, stop=True,
                            )
                        ptt = ptp.tile([128, S], bf16, name="pt")
                        for hf in range(2):
                            hsl = slice(hf * 1024, (hf + 1) * 1024)
                            nc.scalar.activation(
                                ptt[:, hsl], spt[:, hsl], AF.Exp,
                                bias=mb[:, t:t + 1], scale=1.0,
                            )
                        for nb in range(S // 512):
                            nsl = slice(nb * 512, (nb + 1) * 512)
                            nc.tensor.matmul(
                                outp[:, nsl], V[:, t, h * 65:h * 65 + 65],
                                ptt[:, nsl], start=(t == 0), stop=(t == NT - 1),
                                skip_group_check=True,
                            )
                    nc.scalar.activation(OT[h][:], outp[:], AF.Copy, bias=0.0)
                    nc.sync.dma_start(out_d.ap()[h * 65:(h + 1) * 65, :], OT[h][:])

    nc.compile()
    return nc


def kernel(x, mask, Wq, bq, Aq, Bq, Wk, bk, Wv, bv, Av, Bv):
    from concourse import bass_utils

    x, mask = np.asarray(x), np.asarray(mask)
    Wq, bq, Aq, Bq = map(np.asarray, (Wq, bq, Aq, Bq))
    Wk, bk, Wv, bv, Av, Bv = map(np.asarray, (Wk, bk, Wv, bv, Av, Bv))
    isc = 1.0 / np.sqrt(np.float32(W))

    in_maps = []
    for core in range(NCORES):
        b, g = core // 4, core % 4
        rows = slice(g * WPC, (g + 1) * WPC)
        Wq_s = (Wq[rows] * isc).astype(np.float32)
        bq_s = (bq[rows] * isc).astype(np.float32)
        Bq_s = (Bq[rows] * (isc * LORA_SCALE)).astype(np.float32)
        Wk_s, bk_s = Wk[rows], bk[rows]
        Wv_s, bv_s = Wv[rows], bv[rows]
        Bv_s = (Bv[rows] * LORA_SCALE).astype(np.float32)
        WA = np.concatenate(
            [Wq_s, Wk_s, Aq, np.zeros((16, D), np.float32), Av], axis=0
        )  # [432, 768]
        bias = np.zeros((128, 4), np.float32)
        bias[:, 0] = bq_s[0:128]
        bias[0:64, 1] = bq_s[128:192]
        bias[:, 2] = bk_s[0:128]
        bias[0:64, 3] = bk_s[128:192]
        mb = (-10000.0 * (1.0 - mask[b].astype(np.float32))).reshape(NT, 128).T
        in_maps.append({
            "xT": _bf(np.ascontiguousarray(x[b].T)),
            "WAT": _bf(np.ascontiguousarray(WA.T)),
            "WvT": _bf(np.ascontiguousarray(Wv_s.T)),
            "BqT": _bf(np.ascontiguousarray(Bq_s.T)),
            "BvT": _bf(np.ascontiguousarray(Bv_s.T)),
            "bias_qk": bias,
            "bv_row": _bf(bv_s.reshape(1, WPC)),
            "mb": np.ascontiguousarray(mb),
            }
        )

    global _last_in_maps
    _last_in_maps = in_maps
    if "nc" not in _cache:
        _cache["nc"] = _build()
    res = bass_utils.run_bass_kernel_spmd(
        _cache["nc"], in_maps, core_ids=list(range(NCORES))
    )
    out = np.empty((B, S, D), np.float32)
    for core in range(NCORES):
        b, g = core // 4, core % 4
        ot = res.results[core]["outT"].reshape(HPC, 65, S)
        for h in range(HPC):
            blk = ot[h, 0:64, :] / ot[h, 64:65, :]
            out[b, :, g * WPC + h * W:(g * WPC) + (h + 1) * W] = blk.T
    return out


def _bf(a):
    import jax.numpy as jnp

    return np.asarray(jnp.asarray(np.asarray(a, np.float32), jnp.bfloat16))
